# revision 1
# baseline (speedup 1.0000x reference)
"""Tensor-parallel causal multi-head attention (RoPE) on 8 TRN2 NeuronCores.

Sharding: heads are split across the 8 cores (16 heads -> 2 heads/core).
wq/wk/wv are split column-wise (by output head), wo row-wise; hidden_states
is replicated.  Each core computes its 2 heads end-to-end (QKV projection,
RoPE, causal attention, output projection) and returns its additive partial
of the full output; the host sums the 8 partials.

Device-side layout choices (all matmuls contract over the partition dim):
  - X^T [HID, B*S] is produced on the host so projections need no on-device
    transposes.  Q and K are computed directly in transposed layout
    Q^T/K^T [d, s] (lhsT = W^T chunk, rhs = X^T chunk), V in normal layout
    [s, d] (lhsT = X^T chunk, rhs = W^T).
  - Scores are computed transposed: S^T[k, q] = (K^T chunk).T @ Q^T, so the
    exp'd probabilities P^T [k, q] feed the O^T = V.T @ P^T matmul directly
    with q as the 512-wide moving dim (full fp32r rate), no transposes.
  - softmax denominators l[q] = sum_k P^T[k, q] come from a ones-column
    matmul accumulated alongside O^T in PSUM; 1/l (fast DVE reciprocal)
    is broadcast across partitions on the idle GpSimd engine.
  - No max-subtraction: scores are O(1) for this problem so exp is safe.
  - RoPE's rotate_half is a partition swap done with two SBUF->SBUF DMAs;
    the sign flip is folded into the host-prepared sin^T (lower half
    negated), and the 1/sqrt(D) score scale is folded into wq.
"""

import math

import numpy as np

import concourse.bass as bass
import concourse.tile as tile
from concourse import bacc, mybir
from concourse.bass_utils import run_bass_kernel_spmd

B, S, HID = 2, 2048, 2048
H, D = 16, 128
NCORES = 8
HPC = H // NCORES  # heads per core
DH = HPC * D  # per-core projection width (256)
NHC = HID // 128  # hid chunks (16)
TS = 512  # s-tile for projections
TQ = 512  # q-tile for attention
NKB = S // 128  # k blocks per sequence (16)
F32 = mybir.dt.float32
F32R = mybir.dt.float32r

LAST_EXEC_TIME_NS = None
_CACHE = {}


def _build_device_program():
    nc = bacc.Bacc(
        "TRN2",
        target_bir_lowering=False,
        debug=False,
        enable_asserts=False,
        num_devices=NCORES,
    )
    xT = nc.dram_tensor("xT", [HID, B * S], F32R, kind="ExternalInput").ap()
    wqT = nc.dram_tensor("wqT", [HID, DH], F32R, kind="ExternalInput").ap()
    wkT = nc.dram_tensor("wkT", [HID, DH], F32R, kind="ExternalInput").ap()
    wvT = nc.dram_tensor("wvT", [HID, DH], F32R, kind="ExternalInput").ap()
    woT = nc.dram_tensor("woT", [DH, HID], F32R, kind="ExternalInput").ap()
    cosT = nc.dram_tensor("cosT", [D, B * S], F32, kind="ExternalInput").ap()
    sinT = nc.dram_tensor("sinT", [D, B * S], F32, kind="ExternalInput").ap()
    out = nc.dram_tensor("out", [B * S, HID], F32, kind="ExternalOutput").ap()

    with tile.TileContext(nc) as tc:
        _emit_kernel(tc, xT, wqT, wkT, wvT, woT, cosT, sinT, out)

    nc.compile()
    return nc


def _emit_kernel(tc, xT, wqT, wkT, wvT, woT, cosT, sinT, out, dbg=None):
    from contextlib import ExitStack

    nc = tc.nc
    with ExitStack() as ctx:
        xTr = xT.rearrange("(hc p) s -> p hc s", p=128)  # [128, 16, B*S]
        wqTr = wqT.rearrange("(hc p) d -> p hc d", p=128)  # [128, 16, DH]
        wkTr = wkT.rearrange("(hc p) d -> p hc d", p=128)
        wvTr = wvT.rearrange("(hc p) d -> p hc d", p=128)
        woTr = woT.rearrange("(wc p) e -> p wc e", p=128)  # [128, HPC, HID]

        const = ctx.enter_context(tc.tile_pool(name="const", bufs=1))
        batchp = ctx.enter_context(tc.tile_pool(name="batchp", bufs=1))
        xtp = ctx.enter_context(tc.tile_pool(name="xtp", bufs=2))
        csp = ctx.enter_context(tc.tile_pool(name="csp", bufs=2))
        tmpp = ctx.enter_context(tc.tile_pool(name="tmpp", bufs=8))
        ptp = ctx.enter_context(tc.tile_pool(name="ptp", bufs=4))
        recp = ctx.enter_context(tc.tile_pool(name="recp", bufs=2))
        psump = ctx.enter_context(tc.tile_pool(name="psump", bufs=8, space="PSUM"))

        # ---- resident constants ----
        # split weight loads per hid-chunk group so the first matmuls only
        # wait for the chunks they read; wo is loaded later (phase C)
        wq_sb = const.tile([128, NHC, DH], F32R)
        wk_sb = const.tile([128, NHC, DH], F32R)
        wv_sb = const.tile([128, NHC, DH], F32R)
        for j in range(8):
            c0, c1 = j * 2, j * 2 + 2
            nc.scalar.dma_start(out=wq_sb[:, c0:c1, :], in_=wqTr[:, c0:c1, :])
            nc.scalar.dma_start(out=wk_sb[:, c0:c1, :], in_=wkTr[:, c0:c1, :])
            nc.scalar.dma_start(out=wv_sb[:, c0:c1, :], in_=wvTr[:, c0:c1, :])
        wo_sb = const.tile([128, HPC, HID], F32R)
        ones_f = const.tile([128, 1], F32)
        nc.vector.memset(ones_f[:], 1.0)
        ones_col = const.tile([128, 1], F32R)
        nc.scalar.copy(ones_col[:], ones_f[:])

        for b in range(B):
            bs = b * S
            # per-batch on-chip tensors (slots shared across batches via tags)
            qt_sb = batchp.tile([128, HPC, S], F32R, tag="qt")  # Q^T (scaled, roped)
            kt_sb = batchp.tile([128, HPC, S], F32R, tag="kt")  # K^T (roped)
            v_sb = batchp.tile([128, NKB * DH], F32R, tag="v")  # V row-blocks
            at_sb = batchp.tile([128, HPC, S], F32R, tag="at")  # attn out (A^T)

            # ---- phase A: QKV projections + RoPE ----
            for st in range(S // TS):
                s0 = st * TS
                psq = [
                    psump.tile([128, TS], F32, tag="big", name=f"psq{h}")
                    for h in range(HPC)
                ]
                psk = [
                    psump.tile([128, TS], F32, tag="big", name=f"psk{h}")
                    for h in range(HPC)
                ]
                psv = [
                    psump.tile([128, TS], F32, tag="big", name=f"psv{sp}")
                    for sp in range(TS // 256)
                ]
                for half in range(2):
                    xt = xtp.tile([128, 8, TS], F32R)
                    for xj in range(2):
                        nc.sync.dma_start(
                            out=xt[:, xj * 4 : xj * 4 + 4, :],
                            in_=xTr[
                                :,
                                half * 8 + xj * 4 : half * 8 + xj * 4 + 4,
                                bs + s0 : bs + s0 + TS,
                            ],
                        )
                    for i in range(8):
                        hc = half * 8 + i
                        first = hc == 0
                        last = hc == NHC - 1
                        for h in range(HPC):
                            nc.tensor.matmul(
                                psq[h][:],
                                lhsT=(wq_sb[:, hc, h * D : (h + 1) * D]),
                                rhs=(xt[:, i, :]),
                                start=first,
                                stop=last,
                            )
                            nc.tensor.matmul(
                                psk[h][:],
                                lhsT=(wk_sb[:, hc, h * D : (h + 1) * D]),
                                rhs=(xt[:, i, :]),
                                start=first,
                                stop=last,
                            )
                        for sp in range(TS // 256):
                            for sblk in range(2):
                                # one accumulation group per PSUM bank:
                                # start=True clears the whole bank, so only
                                # the first matmul touching the tile starts
                                nc.tensor.matmul(
                                    psv[sp][:, sblk * DH : (sblk + 1) * DH],
                                    lhsT=(
                                        xt[:, i, (sp * 2 + sblk) * 128 : (sp * 2 + sblk + 1) * 128]
                                    ),
                                    rhs=(wv_sb[:, hc, :]),
                                    start=first and sblk == 0,
                                    stop=last and sblk == 1,
                                    skip_group_check=True,
                                )
                # V: evacuate PSUM -> v_sb
                for sp in range(TS // 256):
                    blk0 = s0 // 128 + sp * 2
                    nc.scalar.copy(
                        v_sb[:, blk0 * DH : (blk0 + 2) * DH], psv[sp][:]
                    )
                # RoPE for Q and K
                cs = csp.tile([128, TS], F32, tag="cs")
                nc.sync.dma_start(out=cs[:], in_=cosT[:, bs + s0 : bs + s0 + TS])
                sn = csp.tile([128, TS], F32, tag="cs")
                nc.sync.dma_start(out=sn[:], in_=sinT[:, bs + s0 : bs + s0 + TS])
                for ps_list, dst in ((psq, qt_sb), (psk, kt_sb)):
                    for h in range(HPC):
                        ps = ps_list[h]
                        tq = tmpp.tile([128, TS], F32, tag="tmp")
                        nc.scalar.copy(tq[:], ps[:])
                        tc_cos = tmpp.tile([128, TS], F32, tag="tmp")
                        nc.vector.tensor_mul(tc_cos[:], ps[:], cs[:])
                        tqs = tmpp.tile([128, TS], F32, tag="tmp")
                        nc.sync.dma_start(out=tqs[0:64, :], in_=tq[64:128, :])
                        nc.sync.dma_start(out=tqs[64:128, :], in_=tq[0:64, :])
                        nc.vector.tensor_mul(tqs[:], tqs[:], sn[:])
                        nc.vector.tensor_add(
                            dst[:, h, s0 : s0 + TS], tc_cos[:], tqs[:]
                        )

            if dbg is not None and b == 0:
                nc.sync.dma_start(out=dbg["dqt"][:], in_=qt_sb[:].bitcast(F32))
                nc.sync.dma_start(out=dbg["dkt"][:], in_=kt_sb[:].bitcast(F32))
                nc.sync.dma_start(out=dbg["dv"][:], in_=v_sb[:].bitcast(F32))

            # ---- phase B: causal attention ----
            if b == 0:
                nc.scalar.dma_start(out=wo_sb[:], in_=woTr[:])
            for qt in range(S // TQ):
                q0 = qt * TQ
                for h in range(HPC):
                    nvis = (q0 + TQ) // 128
                    pso = psump.tile([128, TQ], F32, tag="big")
                    psl = psump.tile([1, TQ], F32, tag="big")

                    def score_block(kb):
                        # trim the moving dim to the causal region (min 256
                        # wide so fp32r stays at full rate)
                        off = max(0, kb * 128 - q0)
                        off = min(off, TQ - 256)
                        W = TQ - off
                        pss = psump.tile([128, TQ], F32, tag="big", name="pss")
                        nc.tensor.matmul(
                            pss[:, 0:W],
                            lhsT=(kt_sb[:, h, kb * 128 : (kb + 1) * 128]),
                            rhs=(qt_sb[:, h, q0 + off : q0 + TQ]),
                            start=True,
                            stop=True,
                        )
                        pt = ptp.tile([128, TQ], F32R, tag="pt", name="pt")
                        nc.scalar.activation(
                            pt[:, 0:W],
                            pss[:, 0:W],
                            func=mybir.ActivationFunctionType.Exp,
                        )
                        if kb * 128 + 127 > q0:
                            # diagonal block: zero future positions
                            nc.gpsimd.affine_select(
                                out=pt[:, 0:W],
                                in_=pt[:, 0:W],
                                pattern=[[1, W]],
                                base=q0 + off - kb * 128,
                                channel_multiplier=-1,
                                compare_op=mybir.AluOpType.is_ge,
                                fill=0.0,
                            )
                        return pt, off, W

                    def av_block(kb, pt, off, W):
                        first = kb == 0
                        last = kb == nvis - 1
                        nc.tensor.matmul(
                            pso[:, off:TQ],
                            lhsT=(v_sb[:, kb * DH + h * D : kb * DH + (h + 1) * D]),
                            rhs=(pt[:, 0:W]),
                            start=first,
                            stop=last,
                            skip_group_check=True,
                        )
                        nc.tensor.matmul(
                            psl[:, off:TQ],
                            lhsT=(ones_col[:]),
                            rhs=(pt[:, 0:W]),
                            start=first,
                            stop=last,
                            skip_group_check=True,
                        )
                        if dbg is not None and b == 0 and h == 0 and qt == 3:
                            nc.sync.dma_start(
                                out=dbg["dpt"][:, kb, 0:W], in_=pt[:, 0:W].bitcast(F32)
                            )
                            if off:
                                nc.gpsimd.memset(dbg["dpt"][:, kb, W:TQ], 0.0)

                    # software pipeline: scores run one k-block ahead of AV
                    # so the exp latency is hidden behind PE work
                    pending = None
                    for kb in range(nvis):
                        cur = (kb, *score_block(kb))
                        if pending is not None:
                            av_block(*pending)
                        pending = cur
                    av_block(*pending)
                    if dbg is not None and b == 0 and h == 0:
                        lrow = tmpp.tile([1, TQ], F32, tag="lrow", bufs=1)
                        nc.vector.tensor_copy(lrow[:], psl[:])
                        nc.sync.dma_start(out=dbg["dl"][:, q0 : q0 + TQ], in_=lrow[:])
                    rec = recp.tile([1, TQ], F32, tag="rec")
                    nc.vector.reciprocal_approx_fast(out=rec[:], in_=psl[:])
                    rb = tmpp.tile([128, TQ], F32, tag="tmp")
                    nc.gpsimd.partition_broadcast(rb[:], rec[:])
                    nc.vector.tensor_mul(at_sb[:, h, q0 : q0 + TQ], pso[:], rb[:])

            if dbg is not None and b == 0:
                nc.sync.dma_start(out=dbg["dat"][:], in_=at_sb[:].bitcast(F32))

            # ---- phase C: output projection (partial over local heads) ----
            for sb in range(S // 128):
                for ep in range(2):
                    psus = [
                        psump.tile([128, 512], F32, tag="big", name=f"psu{eu}")
                        for eu in range(2)
                    ]
                    for h in range(HPC):
                        for eu in range(2):
                            et = ep * 2 + eu
                            nc.tensor.matmul(
                                psus[eu][:],
                                lhsT=(at_sb[:, h, sb * 128 : (sb + 1) * 128]),
                                rhs=(wo_sb[:, h, et * 512 : (et + 1) * 512]),
                                start=h == 0,
                                stop=h == HPC - 1,
                            )
                    for eu in range(2):
                        et = ep * 2 + eu
                        ub = tmpp.tile([128, 512], F32, tag="tmp")
                        nc.scalar.copy(ub[:, 0:256], psus[eu][:, 0:256])
                        nc.vector.tensor_copy(ub[:, 256:512], psus[eu][:, 256:512])
                        nc.sync.dma_start(
                            out=out[
                                bs + sb * 128 : bs + (sb + 1) * 128,
                                et * 512 : (et + 1) * 512,
                            ],
                            in_=ub[:],
                        )


def _host_inputs(hidden_states, cos, sin, wq, wk, wv, wo):
    x = np.ascontiguousarray(np.asarray(hidden_states, dtype=np.float32)).reshape(
        B * S, HID
    )
    xT = np.ascontiguousarray(x.T)
    cos = np.asarray(cos, dtype=np.float32)
    sin = np.asarray(sin, dtype=np.float32)
    # [D, B*S], column b*S+s = cos[b, s, :]
    cosT = np.ascontiguousarray(cos.reshape(B * S, D).T)
    sinT = np.ascontiguousarray(sin.reshape(B * S, D).T)
    sinT[: D // 2, :] *= -1.0  # fold rotate_half's negation into sin
    wq = np.asarray(wq, dtype=np.float32)
    wk = np.asarray(wk, dtype=np.float32)
    wv = np.asarray(wv, dtype=np.float32)
    wo = np.asarray(wo, dtype=np.float32)
    scale = 1.0 / math.sqrt(D)
    in_maps = []
    for c in range(NCORES):
        sl = slice(c * DH, (c + 1) * DH)
        in_maps.append(
            {
                "xT": xT,
                "wqT": np.ascontiguousarray(wq[sl].T * scale),
                "wkT": np.ascontiguousarray(wk[sl].T),
                "wvT": np.ascontiguousarray(wv[sl].T),
                "woT": np.ascontiguousarray(wo[:, sl].T),
                "cosT": cosT,
                "sinT": sinT,
            }
        )
    return in_maps


def kernel(
    hidden_states,
    cos,
    sin,
    wq,
    wk,
    wv,
    wo,
    position_ids=None,
    _trace=False,
    _tmpdir=None,
):
    global LAST_EXEC_TIME_NS
    if "nc" not in _CACHE:
        _CACHE["nc"] = _build_device_program()
    nc = _CACHE["nc"]
    in_maps = _host_inputs(hidden_states, cos, sin, wq, wk, wv, wo)
    res = run_bass_kernel_spmd(
        nc,
        in_maps,
        list(range(NCORES)),
        trace=_trace,
        tmpdir=_tmpdir,
    )
    LAST_EXEC_TIME_NS = res.exec_time_ns
    total = res.results[0]["out"].astype(np.float64)
    for c in range(1, NCORES):
        total += res.results[c]["out"]
    return total.astype(np.float32).reshape(B, S, HID)



# revision 3
# speedup vs baseline: 1.0571x; 1.0571x over previous
"""Tensor-parallel causal multi-head attention (RoPE) on 8 TRN2 NeuronCores.

Sharding: 2-way batch x 4-way heads.  Core c handles batch c//4 and heads
[4*(c%4), 4*(c%4)+4).  Each core computes its 4 heads end-to-end for its
batch and writes a bf16 additive partial of that batch's output; the host
sums the 4 partials per batch.

Kernel layout / dtype choices (all-bf16; fp8 was tested and fails the
2e-2 tolerance through the softmax-score path):
  - Everything runs in bf16: projections, scores (Q^T/K^T stored bf16
    after RoPE in fp32 PSUM), probabilities, AV, output projection, and
    the DMA'd output partial.  bf16 matmuls run at full PE rate with
    cheap (FWL) weight loads, vs fp32r's ~187ns serial LDWEIGHTS.
  - x is streamed once per 512-column s-tile and shared by the Q, K and
    V projections of that tile (one 8MB read total).
  - RoPE's rotate-half is a single DVE stream_shuffle: the head dim is
    permuted across partitions (host-side, applied to wq/wk columns and
    cos/sin rows) so each rotate pair sits 16 partitions apart within a
    32-partition quadrant, which STREAM_SHUFFLE can swap (it only
    permutes within 32-partition quadrants).  The rotate sign flip is
    folded into the host-prepared sin.
  - Scores are computed transposed, S^T[k,q], per 2-k-block PSUM group
    [128, 1024] so one scalar-engine exp instruction covers 2 blocks
    (amortizes the 352-cycle ACT overhead).  Softmax denominators come
    from a ones-column bf16 matmul accumulated alongside O^T in PSUM.
  - 1/sqrt(D) is folded into wq on the host; no max-subtraction (scores
    are O(1) so exp is safe).
"""

import math

import numpy as np
import ml_dtypes

import concourse.bass as bass
import concourse.tile as tile
from concourse import bacc, mybir
from concourse.bass_utils import run_bass_kernel_spmd

B, S, HID = 2, 2048, 2048
H, D = 16, 128
NCORES = 8
BGROUP = 4  # cores per batch
HPC = H // BGROUP  # heads per core (4)
DH = HPC * D  # per-core projection width (512)
NHC = HID // 128  # hid chunks (16)
TQ = 512  # q-tile for attention
NKB = S // 128  # k blocks per sequence (16)
F32 = mybir.dt.float32
BF16 = mybir.dt.bfloat16

# stream_shuffle mask: swap 16-partition halves within each 32-part quadrant
SWAP16 = [(i + 16) % 32 for i in range(32)]

# partition permutation: partition 32*q + i holds head-dim
#   d = 16*q + i          (i < 16,  lower half d in [0, 64))
#   d = 64 + 16*q + i-16  (i >= 16, upper half)
# so the rotate-half partner (d <-> d+64) is 16 partitions away in-quadrant.
PERM = np.zeros(128, dtype=np.int64)
for _q in range(4):
    for _i in range(16):
        PERM[32 * _q + _i] = 16 * _q + _i
        PERM[32 * _q + 16 + _i] = 64 + 16 * _q + _i

LAST_EXEC_TIME_NS = None
_CACHE = {}


def _build_device_program():
    nc = bacc.Bacc(
        "TRN2",
        target_bir_lowering=False,
        debug=False,
        enable_asserts=False,
        num_devices=NCORES,
    )
    x16 = nc.dram_tensor("x16", [HID, S], BF16, kind="ExternalInput").ap()
    wq16 = nc.dram_tensor("wq16", [HID, DH], BF16, kind="ExternalInput").ap()
    wk16 = nc.dram_tensor("wk16", [HID, DH], BF16, kind="ExternalInput").ap()
    wv16 = nc.dram_tensor("wv16", [HID, DH], BF16, kind="ExternalInput").ap()
    wo16 = nc.dram_tensor("wo16", [DH, HID], BF16, kind="ExternalInput").ap()
    cs16 = nc.dram_tensor("cs16", [D, S], BF16, kind="ExternalInput").ap()
    sn16 = nc.dram_tensor("sn16", [D, S], BF16, kind="ExternalInput").ap()
    out = nc.dram_tensor("out", [S, HID], BF16, kind="ExternalOutput").ap()

    with tile.TileContext(nc) as tc:
        _emit_kernel(tc, x16, wq16, wk16, wv16, wo16, cs16, sn16, out)

    nc.compile()
    return nc


def _emit_kernel(tc, x16, wq16, wk16, wv16, wo16, cs16, sn16, out):
    from contextlib import ExitStack

    nc = tc.nc
    with ExitStack() as ctx:
        x16r = x16.rearrange("(c p) s -> p c s", p=128)  # [128, 16, S] bf16
        wq16r = wq16.rearrange("(c p) d -> p c d", p=128)  # [128, 16, DH]
        wk16r = wk16.rearrange("(c p) d -> p c d", p=128)
        wv16r = wv16.rearrange("(c p) d -> p c d", p=128)
        wo16r = wo16.rearrange("(h p) e -> p h e", p=128)  # [128, HPC, HID]

        const = ctx.enter_context(tc.tile_pool(name="const", bufs=1))
        seqp = ctx.enter_context(tc.tile_pool(name="seqp", bufs=1))
        xvp = ctx.enter_context(tc.tile_pool(name="xvp", bufs=2))
        ropep = ctx.enter_context(tc.tile_pool(name="ropep", bufs=3))
        ptp = ctx.enter_context(tc.tile_pool(name="ptp", bufs=3))
        atp = ctx.enter_context(tc.tile_pool(name="atp", bufs=2))
        recp = ctx.enter_context(tc.tile_pool(name="recp", bufs=2))
        obp = ctx.enter_context(tc.tile_pool(name="obp", bufs=4))
        psump = ctx.enter_context(tc.tile_pool(name="psump", bufs=2, space="PSUM"))

        # ---- resident inputs ----
        wq16_sb = const.tile([128, NHC, DH], BF16)
        wk16_sb = const.tile([128, NHC, DH], BF16)
        wv16_sb = const.tile([128, NHC, DH], BF16)
        wo16_sb = const.tile([128, HPC, HID], BF16)
        cs_sb = const.tile([128, S], BF16)
        sn_sb = const.tile([128, S], BF16)
        ones16 = const.tile([128, 1], BF16)

        # load order matters: first s-tile needs wk/wv/wq + cos/sin
        for j in range(2):
            nc.scalar.dma_start(
                out=wk16_sb[:, j * 8 : j * 8 + 8, :], in_=wk16r[:, j * 8 : j * 8 + 8, :]
            )
        nc.sync.dma_start(out=cs_sb[:], in_=cs16[:])
        nc.sync.dma_start(out=sn_sb[:], in_=sn16[:])
        for j in range(2):
            nc.scalar.dma_start(
                out=wv16_sb[:, j * 8 : j * 8 + 8, :], in_=wv16r[:, j * 8 : j * 8 + 8, :]
            )
            nc.scalar.dma_start(
                out=wq16_sb[:, j * 8 : j * 8 + 8, :], in_=wq16r[:, j * 8 : j * 8 + 8, :]
            )
        for j in range(2):
            nc.scalar.dma_start(
                out=wo16_sb[:, j * 2 : j * 2 + 2, :], in_=wo16r[:, j * 2 : j * 2 + 2, :]
            )
        nc.vector.memset(ones16[:], 1.0)

        # per-sequence on-chip tensors
        kt16 = seqp.tile([128, HPC, S], BF16)  # K^T roped (perm'd head dim)
        qt16 = seqp.tile([128, HPC, S], BF16)  # Q^T roped+scaled (perm'd)
        v16 = seqp.tile([128, NKB, DH], BF16)  # V row-blocks [k, kb, h*D+d]

        def rope_evac(ps_slice, dst_slice, ss):
            """dst = ps*cos + shuffle16(ps)*sin_folded, for one [128,512]."""
            sh = ropep.tile([128, 512], F32, tag="sh", name="sh")
            nc.vector.stream_shuffle(sh[:], ps_slice, mask=SWAP16)
            t1 = ropep.tile([128, 512], BF16, tag="t1", name="t1")
            nc.gpsimd.tensor_mul(t1[:], sh[:], sn_sb[:, ss : ss + 512])
            t2 = ropep.tile([128, 512], BF16, tag="t2", name="t2")
            nc.vector.tensor_mul(t2[:], ps_slice, cs_sb[:, ss : ss + 512])
            nc.gpsimd.tensor_add(dst_slice, t2[:], t1[:])

        # ---- projections: one x s-tile feeds K, V and Q ----
        for st in range(S // 512):
            s0 = st * 512
            xv = xvp.tile([128, NHC, 512], BF16, tag="xv", name="xv")
            nc.sync.dma_start(out=xv[:], in_=x16r[:, :, s0 : s0 + 512])
            for w_sb, dst in ((wk16_sb, kt16), (wq16_sb, qt16)):
                for pair in range(2):
                    ps = psump.tile([128, 1024], F32, tag="big", name="pskq")
                    for c in range(NHC):
                        for hh in range(2):
                            h = pair * 2 + hh
                            nc.tensor.matmul(
                                ps[:, hh * 512 : (hh + 1) * 512],
                                lhsT=w_sb[:, c, h * 128 : (h + 1) * 128],
                                rhs=xv[:, c, :],
                                start=(c == 0),
                                stop=(c == NHC - 1),
                                skip_group_check=True,
                            )
                    for hh in range(2):
                        h = pair * 2 + hh
                        rope_evac(
                            ps[:, hh * 512 : (hh + 1) * 512],
                            dst[:, h, s0 : s0 + 512],
                            s0,
                        )
                if w_sb is wk16_sb:
                    # V projection for this s-tile (row-block layout)
                    for kb4 in range(4):
                        kb = st * 4 + kb4
                        pv = psump.tile([128, 512], F32, tag="acc", name="pv")
                        for c in range(NHC):
                            nc.tensor.matmul(
                                pv[:],
                                lhsT=xv[:, c, kb4 * 128 : (kb4 + 1) * 128],
                                rhs=wv16_sb[:, c, :],
                                start=(c == 0),
                                stop=(c == NHC - 1),
                            )
                        if kb % 2 == 0:
                            nc.scalar.copy(v16[:, kb, :], pv[:])
                        else:
                            nc.vector.tensor_copy(v16[:, kb, :], pv[:])

        # ---- attention + output projection, per q-tile ----
        for qt in range(S // TQ):
            q0 = qt * TQ
            nvis = (q0 + TQ) // 128
            ngrp = nvis // 2
            atq = atp.tile([128, HPC, TQ], BF16, tag="atq", name="atq")
            for h in range(HPC):
                pso = psump.tile([128, TQ], F32, tag="acc", name="pso")
                psl = psump.tile([1, TQ], F32, tag="psl", name="psl")

                def score_group(gi):
                    # scores for k-blocks (2*gi, 2*gi+1), exp'd into one
                    # bf16 tile; moving dim trimmed to the causal region
                    pss = psump.tile([128, 1024], F32, tag="big", name="pss")
                    offs = []
                    for t in range(2):
                        kb = gi * 2 + t
                        off = max(0, kb * 128 - q0)
                        offs.append(off)
                        nc.tensor.matmul(
                            pss[:, t * 512 + off : t * 512 + 512],
                            lhsT=kt16[:, h, kb * 128 : (kb + 1) * 128],
                            rhs=qt16[:, h, q0 + off : q0 + TQ],
                            start=True,
                            stop=True,
                            skip_group_check=True,
                        )
                    lo = offs[0]
                    pt = ptp.tile([128, 1024], BF16, tag="pt", name="pt")
                    nc.scalar.activation(
                        pt[:, lo:1024],
                        pss[:, lo:1024],
                        func=mybir.ActivationFunctionType.Exp,
                    )
                    for t in range(2):
                        kb = gi * 2 + t
                        off = offs[t]
                        if kb * 128 + 127 > q0:
                            W = TQ - off
                            nc.gpsimd.affine_select(
                                out=pt[:, t * 512 + off : t * 512 + 512],
                                in_=pt[:, t * 512 + off : t * 512 + 512],
                                pattern=[[1, W]],
                                base=q0 + off - kb * 128,
                                channel_multiplier=-1,
                                compare_op=mybir.AluOpType.is_ge,
                                fill=0.0,
                            )
                    return pt, offs

                def av_group(gi, pt, offs):
                    for t in range(2):
                        kb = gi * 2 + t
                        off = offs[t]
                        first = kb == 0
                        last = kb == nvis - 1
                        nc.tensor.matmul(
                            pso[:, off:TQ],
                            lhsT=v16[:, kb, h * D : (h + 1) * D],
                            rhs=pt[:, t * 512 + off : t * 512 + 512],
                            start=first,
                            stop=last,
                            skip_group_check=True,
                        )
                        nc.tensor.matmul(
                            psl[:, off:TQ],
                            lhsT=ones16[:],
                            rhs=pt[:, t * 512 + off : t * 512 + 512],
                            start=first,
                            stop=last,
                            skip_group_check=True,
                        )

                # software pipeline: score group gi+1 issues before AV of gi
                pending = None
                for gi in range(ngrp):
                    cur = (gi, *score_group(gi))
                    if pending is not None:
                        av_group(*pending)
                    pending = cur
                av_group(*pending)

                rec = recp.tile([1, TQ], F32, tag="rec", name="rec")
                nc.vector.reciprocal_approx_fast(out=rec[:], in_=psl[:])
                rb = recp.tile([128, TQ], F32, tag="rb", name="rb")
                nc.gpsimd.partition_broadcast(rb[:], rec[:])
                nc.vector.tensor_mul(atq[:, h, :], pso[:], rb[:])

            # output projection for this q-tile (partial over local heads)
            for sb in range(TQ // 128):
                r0 = q0 + sb * 128
                for ep in range(2):
                    pcs = [
                        psump.tile([128, 512], F32, tag="acc", name=f"pc{eu}")
                        for eu in range(2)
                    ]
                    for h in range(HPC):
                        for eu in range(2):
                            et = ep * 2 + eu
                            nc.tensor.matmul(
                                pcs[eu][:],
                                lhsT=atq[:, h, sb * 128 : (sb + 1) * 128],
                                rhs=wo16_sb[:, h, et * 512 : (et + 1) * 512],
                                start=(h == 0),
                                stop=(h == HPC - 1),
                            )
                    for eu in range(2):
                        et = ep * 2 + eu
                        ob = obp.tile([128, 512], BF16, tag="ob", name="ob")
                        if eu == 0:
                            nc.scalar.copy(ob[:], pcs[eu][:])
                        else:
                            nc.vector.tensor_copy(ob[:], pcs[eu][:])
                        nc.sync.dma_start(
                            out=out[r0 : r0 + 128, et * 512 : (et + 1) * 512],
                            in_=ob[:],
                        )


def _host_inputs(hidden_states, cos, sin, wq, wk, wv, wo):
    bf = ml_dtypes.bfloat16
    x = np.asarray(hidden_states, dtype=np.float32)  # [B, S, HID]
    cos = np.asarray(cos, dtype=np.float32)  # [B, S, D]
    sin = np.asarray(sin, dtype=np.float32)
    wq = np.asarray(wq, dtype=np.float32)
    wk = np.asarray(wk, dtype=np.float32)
    wv = np.asarray(wv, dtype=np.float32)
    wo = np.asarray(wo, dtype=np.float32)
    scale = 1.0 / math.sqrt(D)

    in_maps = []
    for c in range(NCORES):
        b = c // BGROUP
        g = c % BGROUP
        sl = slice(g * DH, (g + 1) * DH)
        xT = np.ascontiguousarray(x[b].T)  # [HID, S]
        # per-head partition permutation of the q/k head dim
        rows = np.concatenate([h * D + PERM for h in range(HPC)])
        wq_sl = wq[sl][rows] * scale  # [DH, HID], rows permuted per head
        wk_sl = wk[sl][rows]
        csT = cos[b].T[PERM]  # [D, S] permuted
        snT = sin[b].T[PERM]
        neg = PERM < 64  # fold rotate_half's negation into sin
        snT = snT * np.where(neg[:, None], -1.0, 1.0).astype(np.float32)
        in_maps.append(
            {
                "x16": xT.astype(bf),
                "wq16": np.ascontiguousarray(wq_sl.T).astype(bf),
                "wk16": np.ascontiguousarray(wk_sl.T).astype(bf),
                "wv16": np.ascontiguousarray(wv[sl].T).astype(bf),
                "wo16": np.ascontiguousarray(wo[:, sl].T).astype(bf),
                "cs16": np.ascontiguousarray(csT).astype(bf),
                "sn16": np.ascontiguousarray(snT).astype(bf),
            }
        )
    return in_maps


def kernel(
    hidden_states,
    cos,
    sin,
    wq,
    wk,
    wv,
    wo,
    position_ids=None,
    _trace=False,
    _tmpdir=None,
):
    global LAST_EXEC_TIME_NS
    if "nc" not in _CACHE:
        _CACHE["nc"] = _build_device_program()
    nc = _CACHE["nc"]
    in_maps = _host_inputs(hidden_states, cos, sin, wq, wk, wv, wo)
    res = run_bass_kernel_spmd(
        nc,
        in_maps,
        list(range(NCORES)),
        trace=_trace,
        tmpdir=_tmpdir,
    )
    LAST_EXEC_TIME_NS = res.exec_time_ns
    full = np.zeros((B, S, HID), dtype=np.float32)
    for c in range(NCORES):
        full[c // BGROUP] += res.results[c]["out"].astype(np.float32)
    return full


# revision 8
# speedup vs baseline: 1.1584x; 1.0958x over previous
"""Tensor-parallel causal multi-head attention (RoPE) on 8 TRN2 NeuronCores.

Sharding: 2-way batch x 4-way heads.  Core c handles batch c//4 and heads
[4*(c%4), 4*(c%4)+4).  Each core computes its 4 heads end-to-end for its
batch and writes a bf16 additive partial of that batch's output; the host
sums the 4 partials per batch.

Kernel layout / dtype choices (all-bf16; fp8 was tested and fails the
2e-2 tolerance through the softmax-score path):
  - Everything runs in bf16: projections, scores (Q^T/K^T stored bf16
    after RoPE in fp32 PSUM), probabilities, AV, output projection, and
    the DMA'd output partial.  bf16 matmuls run at full PE rate with
    cheap (FWL) weight loads, vs fp32r's ~187ns serial LDWEIGHTS.
  - x is streamed once per 512-column s-tile and shared by the Q, K and
    V projections of that tile (one 8MB read total).
  - RoPE's rotate-half is a single DVE stream_shuffle: the head dim is
    permuted across partitions (host-side, applied to wq/wk columns and
    cos/sin rows) so each rotate pair sits 16 partitions apart within a
    32-partition quadrant, which STREAM_SHUFFLE can swap (it only
    permutes within 32-partition quadrants).  The rotate sign flip is
    folded into the host-prepared sin.
  - Scores are computed transposed, S^T[k,q], per 2-k-block PSUM group
    [128, 1024] so one scalar-engine exp instruction covers 2 blocks
    (amortizes the 352-cycle ACT overhead).  Softmax denominators come
    from a ones-column bf16 matmul accumulated alongside O^T in PSUM.
  - 1/sqrt(D) is folded into wq on the host; no max-subtraction (scores
    are O(1) so exp is safe).
"""

import math

import numpy as np
import ml_dtypes

import concourse.bass as bass
import concourse.tile as tile
from concourse import bacc, mybir
from concourse.bass_utils import run_bass_kernel_spmd

B, S, HID = 2, 2048, 2048
H, D = 16, 128
NCORES = 8
BGROUP = 4  # cores per batch
HPC = H // BGROUP  # heads per core (4)
DH = HPC * D  # per-core projection width (512)
NHC = HID // 128  # hid chunks (16)
TQ = 512  # q-tile for attention
NKB = S // 128  # k blocks per sequence (16)
F32 = mybir.dt.float32
BF16 = mybir.dt.bfloat16

# stream_shuffle mask: swap 16-partition halves within each 32-part quadrant
SWAP16 = [(i + 16) % 32 for i in range(32)]

# partition permutation: partition 32*q + i holds head-dim
#   d = 16*q + i          (i < 16,  lower half d in [0, 64))
#   d = 64 + 16*q + i-16  (i >= 16, upper half)
# so the rotate-half partner (d <-> d+64) is 16 partitions away in-quadrant.
PERM = np.zeros(128, dtype=np.int64)
for _q in range(4):
    for _i in range(16):
        PERM[32 * _q + _i] = 16 * _q + _i
        PERM[32 * _q + 16 + _i] = 64 + 16 * _q + _i

LAST_EXEC_TIME_NS = None
_CACHE = {}


def _build_device_program():
    nc = bacc.Bacc(
        "TRN2",
        target_bir_lowering=False,
        debug=False,
        enable_asserts=False,
        num_devices=NCORES,
    )
    x16 = nc.dram_tensor("x16", [HID, S], BF16, kind="ExternalInput").ap()
    wq16 = nc.dram_tensor("wq16", [HID, DH], BF16, kind="ExternalInput").ap()
    wk16 = nc.dram_tensor("wk16", [HID, DH], BF16, kind="ExternalInput").ap()
    wv16 = nc.dram_tensor("wv16", [HID, DH], BF16, kind="ExternalInput").ap()
    wo16 = nc.dram_tensor("wo16", [DH, HID], BF16, kind="ExternalInput").ap()
    cs16 = nc.dram_tensor("cs16", [D, S], BF16, kind="ExternalInput").ap()
    sn16 = nc.dram_tensor("sn16", [D, S], BF16, kind="ExternalInput").ap()
    out = nc.dram_tensor("out", [S, HID], BF16, kind="ExternalOutput").ap()

    with tile.TileContext(nc) as tc:
        _emit_kernel(tc, x16, wq16, wk16, wv16, wo16, cs16, sn16, out)

    nc.compile()
    return nc


def _emit_kernel(tc, x16, wq16, wk16, wv16, wo16, cs16, sn16, out):
    from contextlib import ExitStack

    nc = tc.nc
    with ExitStack() as ctx:
        x16r = x16.rearrange("(c p) s -> p c s", p=128)  # [128, 16, S] bf16
        wq16r = wq16.rearrange("(c p) d -> p c d", p=128)  # [128, 16, DH]
        wk16r = wk16.rearrange("(c p) d -> p c d", p=128)
        wv16r = wv16.rearrange("(c p) d -> p c d", p=128)
        wo16r = wo16.rearrange("(h p) e -> p h e", p=128)  # [128, HPC, HID]

        const = ctx.enter_context(tc.tile_pool(name="const", bufs=1))
        seqp = ctx.enter_context(tc.tile_pool(name="seqp", bufs=1))
        xvp = ctx.enter_context(tc.tile_pool(name="xvp", bufs=2))
        ropep = ctx.enter_context(tc.tile_pool(name="ropep", bufs=3))
        ptp = ctx.enter_context(tc.tile_pool(name="ptp", bufs=4))
        atp = ctx.enter_context(tc.tile_pool(name="atp", bufs=2))
        recp = ctx.enter_context(tc.tile_pool(name="recp", bufs=2))
        obp = ctx.enter_context(tc.tile_pool(name="obp", bufs=4))
        psump = ctx.enter_context(tc.tile_pool(name="psump", bufs=2, space="PSUM"))

        # ---- resident inputs ----
        wq16_sb = const.tile([128, NHC, DH], BF16)
        wk16_sb = const.tile([128, NHC, DH], BF16)
        wv16_sb = const.tile([128, NHC, DH], BF16)
        wo16_sb = const.tile([128, HPC, HID], BF16)
        cs_sb = const.tile([128, S], BF16)
        sn_sb = const.tile([128, S], BF16)
        ones16 = const.tile([128, 1], BF16)

        # load order matters: K-proj consumes wk chunk c at ~0.43*c us, so
        # feed the scalar queue progressively; cos/sin ride the vector DGE
        # so the first RoPE evac (~7us) isn't stuck behind weight loads.
        for c0, c1 in ((0, 2), (2, 4), (4, 10), (10, 16)):
            nc.scalar.dma_start(out=wk16_sb[:, c0:c1, :], in_=wk16r[:, c0:c1, :])
        # first RoPE evac (~7us) only needs the first 512 columns of cos/sin
        nc.scalar.dma_start(out=cs_sb[:, 0:512], in_=cs16[:, 0:512])
        nc.scalar.dma_start(out=sn_sb[:, 0:512], in_=sn16[:, 0:512])
        for j in range(2):
            nc.scalar.dma_start(
                out=wv16_sb[:, j * 8 : j * 8 + 8, :], in_=wv16r[:, j * 8 : j * 8 + 8, :]
            )
        nc.scalar.dma_start(out=cs_sb[:, 512:S], in_=cs16[:, 512:S])
        nc.scalar.dma_start(out=sn_sb[:, 512:S], in_=sn16[:, 512:S])
        for j in range(2):
            nc.scalar.dma_start(
                out=wq16_sb[:, j * 8 : j * 8 + 8, :], in_=wq16r[:, j * 8 : j * 8 + 8, :]
            )
        for j in range(2):
            nc.scalar.dma_start(
                out=wo16_sb[:, j * 2 : j * 2 + 2, :], in_=wo16r[:, j * 2 : j * 2 + 2, :]
            )
        nc.vector.memset(ones16[:], 1.0)

        # per-sequence on-chip tensors
        kt16 = seqp.tile([128, HPC, S], BF16)  # K^T roped (perm'd head dim)
        qt16 = seqp.tile([128, HPC, S], BF16)  # Q^T roped+scaled (perm'd)
        v16 = seqp.tile([128, NKB, DH], BF16)  # V row-blocks [k, kb, h*D+d]

        def rope_evac(ps_slice, dst_slice, ss):
            """dst = ps*cos + shuffle16(ps)*sin_folded, for one [128,512]."""
            sh = ropep.tile([128, 512], F32, tag="sh", name="sh")
            nc.vector.stream_shuffle(sh[:], ps_slice, mask=SWAP16)
            t1 = ropep.tile([128, 512], BF16, tag="t1", name="t1")
            nc.gpsimd.tensor_mul(t1[:], sh[:], sn_sb[:, ss : ss + 512])
            t2 = ropep.tile([128, 512], BF16, tag="t2", name="t2")
            nc.vector.tensor_mul(t2[:], ps_slice, cs_sb[:, ss : ss + 512])
            nc.gpsimd.tensor_add(dst_slice, t2[:], t1[:])

        # ---- projections: one x s-tile feeds K, V and Q ----
        for st in range(S // 512):
            s0 = st * 512
            xv = xvp.tile([128, NHC, 512], BF16, tag="xv", name="xv")
            if st == 0:
                # progressive pieces so K-proj's c-loop starts early
                for c0, c1 in ((0, 2), (2, 4), (4, 10), (10, 16)):
                    nc.sync.dma_start(
                        out=xv[:, c0:c1, :], in_=x16r[:, c0:c1, s0 : s0 + 512]
                    )
            else:
                nc.sync.dma_start(out=xv[:], in_=x16r[:, :, s0 : s0 + 512])
            for w_sb, dst in ((wk16_sb, kt16), (wq16_sb, qt16)):
                for pair in range(2):
                    ps = psump.tile([128, 1024], F32, tag="big", name="pskq")
                    for c in range(NHC):
                        for hh in range(2):
                            h = pair * 2 + hh
                            nc.tensor.matmul(
                                ps[:, hh * 512 : (hh + 1) * 512],
                                lhsT=w_sb[:, c, h * 128 : (h + 1) * 128],
                                rhs=xv[:, c, :],
                                start=(c == 0),
                                stop=(c == NHC - 1),
                                skip_group_check=True,
                            )
                    for hh in range(2):
                        h = pair * 2 + hh
                        rope_evac(
                            ps[:, hh * 512 : (hh + 1) * 512],
                            dst[:, h, s0 : s0 + 512],
                            s0,
                        )
                if w_sb is wk16_sb:
                    # V projection for this s-tile (row-block layout)
                    for kb4 in range(4):
                        kb = st * 4 + kb4
                        pv = psump.tile([128, 512], F32, tag="acc", name="pv")
                        for c in range(NHC):
                            nc.tensor.matmul(
                                pv[:],
                                lhsT=xv[:, c, kb4 * 128 : (kb4 + 1) * 128],
                                rhs=wv16_sb[:, c, :],
                                start=(c == 0),
                                stop=(c == NHC - 1),
                            )
                        if kb % 2 == 0:
                            nc.scalar.copy(v16[:, kb, :], pv[:])
                        else:
                            nc.vector.tensor_copy(v16[:, kb, :], pv[:])

        # ---- attention + output projection ----
        # Flat software pipeline over (qt, h, group) tokens with LOOK score
        # groups in flight, continuing across head and q-tile boundaries so
        # the scalar-engine exp latency stays hidden.  The out-projection of
        # q-tile qt is emitted right after its last head normalizes, with
        # the next q-tile's first score groups already issued so their exps
        # run on ACT while the PE does the out-projection matmuls.
        LOOK = 2
        ngrp_of = lambda qt: (qt + 1) * 2

        def score_group(qt, h, gi):
            # scores for k-blocks (2*gi, 2*gi+1), exp'd into one bf16
            # tile; moving dim trimmed to the causal region
            q0 = qt * TQ
            pss = psump.tile([128, 1024], F32, tag="big", name="pss")
            offs = []
            for t in range(2):
                kb = gi * 2 + t
                off = max(0, kb * 128 - q0)
                offs.append(off)
                nc.tensor.matmul(
                    pss[:, t * 512 + off : t * 512 + 512],
                    lhsT=kt16[:, h, kb * 128 : (kb + 1) * 128],
                    rhs=qt16[:, h, q0 + off : q0 + TQ],
                    start=True,
                    stop=True,
                    skip_group_check=True,
                )
            lo = offs[0]
            pt = ptp.tile([128, 1024], BF16, tag="pt", name="pt")
            nc.scalar.activation(
                pt[:, lo:1024],
                pss[:, lo:1024],
                func=mybir.ActivationFunctionType.Exp,
            )
            for t in range(2):
                kb = gi * 2 + t
                off = offs[t]
                if kb * 128 + 127 > q0:
                    W = TQ - off
                    nc.gpsimd.affine_select(
                        out=pt[:, t * 512 + off : t * 512 + 512],
                        in_=pt[:, t * 512 + off : t * 512 + 512],
                        pattern=[[1, W]],
                        base=q0 + off - kb * 128,
                        channel_multiplier=-1,
                        compare_op=mybir.AluOpType.is_ge,
                        fill=0.0,
                    )
            return pt, offs

        atqs = {}  # qt -> atq tile
        accs = {}  # (qt, h) -> (pso, psl)

        def flush_one(pend):
            qt, h, gi, pt, offs = pend.pop(0)
            nvis = ngrp_of(qt) * 2
            if (qt, h) not in accs:
                pso = psump.tile([128, TQ], F32, tag="acc", name="pso")
                psl = psump.tile([1, TQ], F32, tag="psl", name="psl")
                accs[(qt, h)] = (pso, psl)
            pso, psl = accs[(qt, h)]
            for t in range(2):
                kb = gi * 2 + t
                off = offs[t]
                first = kb == 0
                last = kb == nvis - 1
                nc.tensor.matmul(
                    pso[:, off:TQ],
                    lhsT=v16[:, kb, h * D : (h + 1) * D],
                    rhs=pt[:, t * 512 + off : t * 512 + 512],
                    start=first,
                    stop=last,
                    skip_group_check=True,
                )
                nc.tensor.matmul(
                    psl[:, off:TQ],
                    lhsT=ones16[:],
                    rhs=pt[:, t * 512 + off : t * 512 + 512],
                    start=first,
                    stop=last,
                    skip_group_check=True,
                )
            if gi == ngrp_of(qt) - 1:
                rec = recp.tile([1, TQ], F32, tag="rec", name="rec")
                nc.vector.reciprocal_approx_fast(out=rec[:], in_=psl[:])
                rb = recp.tile([128, TQ], F32, tag="rb", name="rb")
                nc.gpsimd.partition_broadcast(rb[:], rec[:])
                nc.vector.tensor_mul(atqs[qt][:, h, :], pso[:], rb[:])
                del accs[(qt, h)]
                if h == HPC - 1:
                    emit_outproj(qt)

        def emit_outproj(qt):
            # partial over local heads; pc tiles rotate through the acc and
            # psl tag slots (4 total) so evacuation latency stays hidden
            q0 = qt * TQ
            pcn = 0
            for sb in range(TQ // 128):
                r0 = q0 + sb * 128
                for ep in range(2):
                    pcs = []
                    for eu in range(2):
                        tag = "acc" if pcn % 2 == 0 else "psl"
                        pcs.append(
                            psump.tile([128, 512], F32, tag=tag, name=f"pc{eu}")
                        )
                        pcn += 1
                    for h in range(HPC):
                        for eu in range(2):
                            et = ep * 2 + eu
                            nc.tensor.matmul(
                                pcs[eu][:],
                                lhsT=atq_c[qt][:, h, sb * 128 : (sb + 1) * 128],
                                rhs=wo16_sb[:, h, et * 512 : (et + 1) * 512],
                                start=(h == 0),
                                stop=(h == HPC - 1),
                            )
                    for eu in range(2):
                        et = ep * 2 + eu
                        ob = obp.tile([128, 512], BF16, tag="ob", name="ob")
                        nc.vector.tensor_copy(ob[:], pcs[eu][:])
                        nc.sync.dma_start(
                            out=out[r0 : r0 + 128, et * 512 : (et + 1) * 512],
                            in_=ob[:],
                        )

        atq_c = atqs  # alias used by emit_outproj
        tokens = [
            (qt, h, gi)
            for qt in range(S // TQ)
            for h in range(HPC)
            for gi in range(ngrp_of(qt))
        ]
        pend = []
        for qt, h, gi in tokens:
            if qt not in atqs:
                atqs[qt] = atp.tile([128, HPC, TQ], BF16, tag="atq", name="atq")
            pend.append((qt, h, gi, *score_group(qt, h, gi)))
            if len(pend) > LOOK:
                flush_one(pend)
        while pend:
            flush_one(pend)


def _host_inputs(hidden_states, cos, sin, wq, wk, wv, wo):
    bf = ml_dtypes.bfloat16
    x = np.asarray(hidden_states, dtype=np.float32)  # [B, S, HID]
    cos = np.asarray(cos, dtype=np.float32)  # [B, S, D]
    sin = np.asarray(sin, dtype=np.float32)
    wq = np.asarray(wq, dtype=np.float32)
    wk = np.asarray(wk, dtype=np.float32)
    wv = np.asarray(wv, dtype=np.float32)
    wo = np.asarray(wo, dtype=np.float32)
    scale = 1.0 / math.sqrt(D)

    in_maps = []
    for c in range(NCORES):
        b = c // BGROUP
        g = c % BGROUP
        sl = slice(g * DH, (g + 1) * DH)
        xT = np.ascontiguousarray(x[b].T)  # [HID, S]
        # per-head partition permutation of the q/k head dim
        rows = np.concatenate([h * D + PERM for h in range(HPC)])
        wq_sl = wq[sl][rows] * scale  # [DH, HID], rows permuted per head
        wk_sl = wk[sl][rows]
        csT = cos[b].T[PERM]  # [D, S] permuted
        snT = sin[b].T[PERM]
        neg = PERM < 64  # fold rotate_half's negation into sin
        snT = snT * np.where(neg[:, None], -1.0, 1.0).astype(np.float32)
        in_maps.append(
            {
                "x16": xT.astype(bf),
                "wq16": np.ascontiguousarray(wq_sl.T).astype(bf),
                "wk16": np.ascontiguousarray(wk_sl.T).astype(bf),
                "wv16": np.ascontiguousarray(wv[sl].T).astype(bf),
                "wo16": np.ascontiguousarray(wo[:, sl].T).astype(bf),
                "cs16": np.ascontiguousarray(csT).astype(bf),
                "sn16": np.ascontiguousarray(snT).astype(bf),
            }
        )
    return in_maps


def kernel(
    hidden_states,
    cos,
    sin,
    wq,
    wk,
    wv,
    wo,
    position_ids=None,
    _trace=False,
    _tmpdir=None,
):
    global LAST_EXEC_TIME_NS
    if "nc" not in _CACHE:
        _CACHE["nc"] = _build_device_program()
    nc = _CACHE["nc"]
    in_maps = _host_inputs(hidden_states, cos, sin, wq, wk, wv, wo)
    res = run_bass_kernel_spmd(
        nc,
        in_maps,
        list(range(NCORES)),
        trace=_trace,
        tmpdir=_tmpdir,
    )
    LAST_EXEC_TIME_NS = res.exec_time_ns
    full = np.zeros((B, S, HID), dtype=np.float32)
    for c in range(NCORES):
        full[c // BGROUP] += res.results[c]["out"].astype(np.float32)
    return full
